# revision 2
# baseline (speedup 1.0000x reference)
"""Trainium2 Bass kernel for GQA multi-head attention (B=2, S=2048, D=2048,
16 Q heads / 4 KV heads, head_dim=128, RoPE, causal). bf16 compute, fp32 accum.

Sharding: 8 cores = 2 (batch) x 4 (tensor-parallel head groups).
Each core: 4 Q heads + 1 KV head for one batch element; partial output
projection [D, S]; host sums the 4 TP partials per batch element.

Per-core dataflow (all bf16 in SBUF, fp32 in PSUM):
  phase 1 (per s-quarter sq, d-chunk dc):
    pQ[i] [128, 512] += wq_chunk.T @ xt      (4 heads)
    pK    [128, 512] += wk_chunk.T @ xt
    pV    [128s,128dv] blocks += xt_slice.T @ wv_chunk   (natural V layout)
    psum -> bf16 sbuf (ACT); RoPE on Q/K via 4 DVE ops using a negated-sin
    table (rows 0:64 hold -sin so one add combines both halves); V blocks
    land in VnO with a ones column appended per 128-row chunk.
  phase 2 (per head h, 512-wide q-group qg):
    scoresT [k, q] chunk pairs -> psum; exp (ACT) -> at bf16 sbuf;
    diagonal 128x128 block masked on Pool (upper-tri 0/1 mul).
    AV flipped: pav[q, 129] += at_block.T @ VnO_chunk[dv | ones]
      -> softmax denominator rides in column 128 for free.
    normalize: aot = pav[:,0:128] * reciprocal(pav[:,128]) (DVE tensor_scalar)
    PE-transpose aot -> AO[h] [dv, q].
  phase 3: outT[Dc, :] = sum_hc wo_chunk.T @ AO[hc]  (psum accum over heads)

Output: per-core outT [D, S] bf16; host sums out[b] = sum_tp outT.T
"""

import numpy as np
from contextlib import ExitStack

import ml_dtypes
import concourse.bass as bass
import concourse.mybir as mybir
from concourse import bacc, tile
from concourse.bass_utils import run_bass_kernel_spmd
from concourse.masks import make_identity, make_upper_triangular

F32 = mybir.dt.float32
BF16 = mybir.dt.bfloat16
AF = mybir.ActivationFunctionType

S = 2048
D = 2048
P = 128
NHQ = 4   # q heads per core
N_CORES = 8
N_TP = 4
VW = 132  # VnO column stride per k-chunk: 128 dv + 1 ones + 3 pad


def _build_kernel(nc, tc, ctx, xT, wq, wkv, wo, cos2, sin2n, outT):
    const = ctx.enter_context(tc.tile_pool(name="const", bufs=1))
    xtp = ctx.enter_context(tc.tile_pool(name="xtp", bufs=6))
    sbp = ctx.enter_context(tc.tile_pool(name="sbp", bufs=6))
    ropep = ctx.enter_context(tc.tile_pool(name="ropep", bufs=4))
    atp = ctx.enter_context(tc.tile_pool(name="atp", bufs=12))
    aotp = ctx.enter_context(tc.tile_pool(name="aotp", bufs=9))
    recp = ctx.enter_context(tc.tile_pool(name="recp", bufs=9))
    obp = ctx.enter_context(tc.tile_pool(name="obp", bufs=4))

    # ---- persistent weights / tables ----
    # interleave wq/wkv chunk loads in the order phase 1 consumes them, on
    # the DVE queue (the ACT queue is blocked early by the act-table load)
    wqc = const.tile([P, 16 * 512], BF16, tag="wqc")
    wkvc = const.tile([P, 16 * 256], BF16, tag="wkvc")
    for dc in range(16):
        qeng = nc.gpsimd if dc % 2 == 0 else nc.scalar
        keng = nc.scalar if dc % 2 == 0 else nc.gpsimd
        qeng.dma_start(
            wqc[:, 512 * dc : 512 * (dc + 1)], wq[128 * dc : 128 * (dc + 1), :]
        )
        keng.dma_start(
            wkvc[:, 256 * dc : 256 * (dc + 1)], wkv[128 * dc : 128 * (dc + 1), :]
        )
    cos2t = const.tile([P, S], BF16, tag="cos2t")
    sin2nt = const.tile([P, S], BF16, tag="sin2nt")
    nc.scalar.dma_start(cos2t[:], cos2[:])
    nc.scalar.dma_start(sin2nt[:], sin2n[:])

    maskt = const.tile([P, P], BF16, tag="maskt")
    make_upper_triangular(nc, maskt[:], val=1.0, diag=True)
    ident = const.tile([P, P], BF16, tag="ident")
    make_identity(nc, ident[:])

    # wo, rearranged: col block hc (2048 wide) = wo[128*hc : +128, :]
    woc = const.tile([P, 4 * D], BF16, tag="woc")
    for hc in range(NHQ):
        nc.scalar.dma_start(woc[:, D * hc : D * (hc + 1)], wo[128 * hc : 128 * (hc + 1), :])

    QT = [const.tile([P, S], BF16, tag=f"QT{i}", name=f"QT{i}") for i in range(NHQ)]
    KT = const.tile([P, S], BF16, tag="KT")
    # VnO: per k-chunk c: cols [VW*c, VW*c+128) = V rows [128c,+128) x dv,
    # col VW*c+128 = ones (softmax denominator rides the AV matmul)
    VnO = const.tile([P, 16 * VW], BF16, tag="VnO")
    for c in range(16):
        nc.gpsimd.memset(VnO[:, VW * c + 128 : VW * c + 129], 1.0)
    AO = [const.tile([P, S], BF16, tag=f"AO{i}", name=f"AO{i}") for i in range(NHQ)]

    # ---- phase 1: projections + RoPE ----
    with tc.tile_pool(name="ps1", bufs=1, space="PSUM") as ps1:
        _phase1(nc, ps1, xtp, sbp, ropep, xT, wqc, wkvc, cos2t, sin2nt, ident, QT, KT, VnO)

    # ---- phases 2+3: attention with interleaved output projection ----
    with tc.tile_pool(name="ps2", bufs=1, space="PSUM") as ps2:
        _phase23(nc, ps2, atp, aotp, recp, obp, maskt, ident, QT, KT, VnO, AO,
                 woc, outT)


def _phase1(nc, ps1, xtp, sbp, ropep, xT, wqc, wkvc, cos2t, sin2nt, ident, QT, KT, VnO):
    pend_v = None  # (vsb, sq): V transposes deferred behind next sq's matmuls

    def flush_v(pend):
        if pend is None:
            return
        vsb, vsq = pend
        vtp_f = ps1.tile([P, 512], F32, tag="acc", bufs=8, name="vtp_f")
        vtp = vtp_f.bitcast(BF16)[:, 0:512]
        for t in range(4):
            nc.tensor.transpose(
                vtp[:, 128 * t : 128 * (t + 1)], vsb[:, 128 * t : 128 * (t + 1)],
                ident[:],
            )
        for t in range(4):
            j = 4 * vsq + t
            nc.scalar.activation(
                VnO[:, VW * j : VW * j + 128], vtp[:, 128 * t : 128 * (t + 1)],
                AF.Copy,
            )

    for sq in range(4):
        s0 = 512 * sq
        sl = slice(s0, s0 + 512)
        pQ = [ps1.tile([P, 512], F32, tag="acc", bufs=8, name=f"pQ{i}") for i in range(NHQ)]
        pK = ps1.tile([P, 512], F32, tag="acc", bufs=8)
        pV = ps1.tile([P, 512], F32, tag="acc", bufs=8)
        for dc in range(16):
            d0 = 128 * dc
            xt = xtp.tile([P, 512], BF16)
            nc.sync.dma_start(xt[:], xT[d0 : d0 + 128, sl])
            st, sp = dc == 0, dc == 15
            for i in range(NHQ):
                nc.tensor.matmul(
                    pQ[i][:], wqc[:, 512 * dc + 128 * i : 512 * dc + 128 * (i + 1)],
                    xt[:], start=st, stop=sp,
                )
            nc.tensor.matmul(
                pK[:], wkvc[:, 256 * dc : 256 * dc + 128], xt[:], start=st, stop=sp
            )
            # V transposed like K: pV [dv, s]
            nc.tensor.matmul(
                pV[:], wkvc[:, 256 * dc + 128 : 256 * dc + 256], xt[:],
                start=st, stop=sp,
            )
            if dc == 1:
                flush_v(pend_v)
                pend_v = None
        # psum -> bf16 sbuf: natural copy + half-swapped copy (partition-
        # shifted copies from PSUM are legal on both ACT and DVE). Drains
        # alternate ACT/DVE so the six banks free ~2x faster for the next
        # sq; RoPE itself is 3 full-width all-bf16 DVE ops (2x mode):
        # dst = qs*cos + swap(qs)*sin_negated
        if sq == 3:
            # last sq: V first so the final flush_v never waits the chain
            vsb = sbp.tile([P, 512], BF16, tag="vsb")
            nc.scalar.activation(vsb[:], pV[:], AF.Copy)
        tensors = [(pQ[i], QT[i]) for i in range(NHQ)] + [(pK, KT)]
        for i, (psrc, dst) in enumerate(tensors):
            qs = sbp.tile([P, 512], BF16, tag="qs")
            qsw = sbp.tile([P, 512], BF16, tag="qsw")
            if i % 2 == 0:
                nc.scalar.activation(qs[:], psrc[:], AF.Copy)
                nc.scalar.activation(qsw[0:64, :], psrc[64:128, :], AF.Copy)
                nc.scalar.activation(qsw[64:128, :], psrc[0:64, :], AF.Copy)
            else:
                nc.vector.tensor_scalar_add(qs[:], psrc[:], 0.0)
                nc.vector.tensor_scalar_add(qsw[0:64, :], psrc[64:128, :], 0.0)
                nc.vector.tensor_scalar_add(qsw[64:128, :], psrc[0:64, :], 0.0)
            m1 = ropep.tile([P, 512], BF16, tag="m1")
            m2 = ropep.tile([P, 512], BF16, tag="m2")
            nc.vector.tensor_mul(m1[:], qs[:], cos2t[:, sl])
            nc.vector.tensor_mul(m2[:], qsw[:], sin2nt[:, sl])
            nc.vector.tensor_add(dst[:, sl], m1[:], m2[:])
        if sq != 3:
            vsb = sbp.tile([P, 512], BF16, tag="vsb")
            nc.scalar.activation(vsb[:], pV[:], AF.Copy)
        pend_v = (vsb, sq)
    flush_v(pend_v)


def _flush_transposes(nc, ps2, ident, AO, pend):
    """Deferred PE transposes of normalized [q, dv] blocks -> AO [dv, q].
    Issued after the NEXT group's score matmuls so the in-order PE never
    stalls on the DVE normalize chain."""
    if pend is None:
        return
    h, items = pend
    ptp = ps2.tile([P, 512], BF16, tag="tp", bufs=1)
    for i, (aot, j) in enumerate(items):
        o = 128 * i
        nc.tensor.transpose(ptp[:, o : o + 128], aot[:], ident[:])
        nc.vector.tensor_scalar_add(
            AO[h][:, 128 * j : 128 * (j + 1)], ptp[:, o : o + 128], 0.0
        )


def _emit_po(nc, ps2, obp, woc, AO, outT, Dc, half, alt):
    """One output-projection series: po[Dcols, 1024q] = sum_hc wo.T @ AO.
    Allocates from the same rotating psum tag as score tiles so phase-3
    series interleave into phase 2 without extra banks."""
    D0 = 128 * Dc
    po = ps2.tile([P, 1024], F32, tag="sc", bufs=3)
    for hc in range(NHQ):
        for js in range(2):
            o0 = 512 * js
            nc.tensor.matmul(
                po[:, o0 : o0 + 512],
                woc[:, D * hc + D0 : D * hc + D0 + 128],
                AO[hc][:, 1024 * half + o0 : 1024 * half + o0 + 512],
                start=hc == 0, stop=hc == 3,
            )
    ob = obp.tile([P, 1024], BF16)
    if alt % 2 == 0:
        nc.scalar.activation(ob[:], po[:], AF.Copy)
    else:
        nc.vector.tensor_scalar_add(ob[:], po[:], 0.0)
    nc.sync.dma_start(outT[D0 : D0 + 128, 1024 * half : 1024 * (half + 1)], ob[:])


def _phase23(nc, ps2, atp, aotp, recp, obp, maskt, ident, QT, KT, VnO, AO, woc, outT):
    pend = None
    po_cnt = 0  # output-projection series issued so far (16 Dc x 2 halves)
    for qg in range(4):
        for h in range(NHQ):
            q0 = 512 * qg
            nchunks = 4 * (qg + 1)  # k-chunks 0..nchunks-1 (diag group last)
            at_tiles = []
            for cp in range(nchunks // 2):
                psc = ps2.tile([P, 1024], F32, tag="sc", bufs=3)
                rels = []
                for ci in range(2):
                    c = 2 * cp + ci
                    rel = max(0, 128 * c - q0)
                    rels.append(rel)
                    nc.tensor.matmul(
                        psc[:, 512 * ci + rel : 512 * (ci + 1)],
                        KT[:, 128 * c : 128 * (c + 1)],
                        QT[h][:, q0 + rel : q0 + 512],
                        start=True, stop=True,
                    )
                at = atp.tile([P, 1024], BF16)
                # exp over [rels[0]:1024] — the unwritten psum gap of the
                # second chunk (cols 512..512+rels[1]) is exp'd harmlessly;
                # those at-blocks are never read (causally skipped).
                nc.scalar.activation(at[:, rels[0] : 1024], psc[:, rels[0] : 1024], AF.Exp)
                at_tiles.append(at)
                # diagonal-block masks (Pool): chunk c == q-block j
                for ci in range(2):
                    c = 2 * cp + ci
                    if 4 * qg <= c <= 4 * qg + 3:
                        jj = c - 4 * qg
                        o = 512 * ci + 128 * jj
                        nc.gpsimd.tensor_mul(
                            at[:, o : o + 128], at[:, o : o + 128], maskt[:]
                        )
            # previous group's transposes ride behind this group's scores
            _flush_transposes(nc, ps2, ident, AO, pend)
            # always-ready output-projection series fill the exp-latency lull
            if qg >= 2:
                _emit_po(nc, ps2, obp, woc, AO, outT, po_cnt, 0, po_cnt)
                po_cnt += 1
            # AV with ones column; accumulate over chunks per q-block j
            items = []
            for jj in range(4):
                j = 4 * qg + jj
                if jj % 2 == 0:
                    pavt = ps2.tile([P, 512], F32, tag="av", bufs=1)
                o = 256 * (jj % 2)
                for c in range(j + 1):
                    at = at_tiles[c // 2]
                    nc.tensor.matmul(
                        pavt[:, o : o + 129],
                        at[:, 512 * (c & 1) + 128 * jj : 512 * (c & 1) + 128 * (jj + 1)],
                        VnO[:, VW * c : VW * c + 129],
                        start=c == 0, stop=c == j,
                        skip_group_check=True,
                    )
                rec = recp.tile([P, 1], F32)
                nc.vector.reciprocal(rec[:], pavt[:, o + 128 : o + 129])
                aot = aotp.tile([P, P], BF16)
                nc.vector.tensor_scalar_mul(aot[:], pavt[:, o : o + 128], rec[:])
                items.append((aot, j))
            pend = (h, items)
            # after the first q-half (qg 0-1) is done for all heads, slide
            # half-0 output-projection series into the remaining groups
            if qg >= 2:
                _emit_po(nc, ps2, obp, woc, AO, outT, po_cnt, 0, po_cnt)
                po_cnt += 1
    _flush_transposes(nc, ps2, ident, AO, pend)
    # remaining output projection (half 1)
    for Dc in range(16):
        _emit_po(nc, ps2, obp, woc, AO, outT, Dc, 1, Dc)


_NC_CACHE = {}


def _get_nc(reps=1):
    if reps in _NC_CACHE:
        return _NC_CACHE[reps]
    nc = bacc.Bacc("TRN2", target_bir_lowering=False, debug=False)
    aps = {}
    for name, shape, dt in [
        ("xT", [D, S], BF16), ("wq", [D, 512], BF16), ("wkv", [D, 2 * P], BF16),
        ("wo", [512, D], BF16), ("cos2", [P, S], BF16), ("sin2n", [P, S], BF16),
    ]:
        aps[name] = nc.dram_tensor(name, shape, dt, kind="ExternalInput").ap()
    outT = nc.dram_tensor("outT", [D, S], BF16, kind="ExternalOutput").ap()
    with tile.TileContext(nc) as tc, ExitStack() as ctx:
        if reps == 1:
            _build_kernel(
                nc, tc, ctx, aps["xT"], aps["wq"], aps["wkv"], aps["wo"],
                aps["cos2"], aps["sin2n"], outT,
            )
        else:
            with tc.For_i(0, reps, 1):
                with ExitStack() as inner:
                    _build_kernel(
                        nc, tc, inner, aps["xT"], aps["wq"], aps["wkv"],
                        aps["wo"], aps["cos2"], aps["sin2n"], outT,
                    )
    nc.compile()
    _NC_CACHE[reps] = nc
    return nc


def _prep_in_maps(x, freqs_cos, freqs_sin, w_q, w_k, w_v, w_o):
    bf = ml_dtypes.bfloat16
    x = np.asarray(x, np.float32)
    cosT = np.asarray(freqs_cos, np.float32).T  # [64, S]
    sinT = np.asarray(freqs_sin, np.float32).T
    cos2 = np.ascontiguousarray(np.concatenate([cosT, cosT], 0)).astype(bf)
    # negated-sin table: rows 0:64 = -sin (imag-half product), 64:128 = +sin
    sin2n = np.ascontiguousarray(np.concatenate([-sinT, sinT], 0)).astype(bf)
    w_q = np.asarray(w_q, np.float32)
    w_k = np.asarray(w_k, np.float32)
    w_v = np.asarray(w_v, np.float32)
    w_o = np.asarray(w_o, np.float32)

    # deinterleave head_dim: evens then odds (consistent for q and k)
    perm1 = np.concatenate([np.arange(0, P, 2), np.arange(1, P, 2)])
    in_maps = []
    for core in range(N_CORES):
        b, tp = divmod(core, N_TP)
        qcols = np.concatenate([4 * tp * P + h * P + perm1 for h in range(NHQ)])
        kcols = tp * P + perm1
        wq_c = np.ascontiguousarray(w_q[:, qcols] * (P ** -0.5)).astype(bf)
        wkv_c = np.ascontiguousarray(np.concatenate(
            [w_k[:, kcols], w_v[:, tp * P : (tp + 1) * P]], axis=1)).astype(bf)
        wo_c = np.ascontiguousarray(w_o[4 * tp * P : 4 * (tp + 1) * P, :]).astype(bf)
        xTc = np.ascontiguousarray(x[b].T).astype(bf)
        in_maps.append({
            "xT": xTc, "wq": wq_c, "wkv": wkv_c, "wo": wo_c,
            "cos2": cos2, "sin2n": sin2n,
        })
    return in_maps


def kernel(x, freqs_cos, freqs_sin, w_q, w_k, w_v, w_o):
    nc = _get_nc()
    in_maps = _prep_in_maps(x, freqs_cos, freqs_sin, w_q, w_k, w_v, w_o)
    results = run_bass_kernel_spmd(nc, in_maps, list(range(N_CORES))).results
    B = 2
    out = np.zeros((B, S, D), np.float32)
    for core in range(N_CORES):
        out[core // N_TP] += results[core]["outT"].astype(np.float32).T
    return out
